# revision 6
# baseline (speedup 1.0000x reference)
"""Trainium2 Bass kernel for multi-head cross-attention.

Reference computation (fp32):
    q = (x @ W_q), k = (ctx @ W_k), v = (ctx @ W_v)   # per-head d=64, 16 heads
    out = softmax(q k^T / 8) v  concat-heads @ W_o + b_o

Sharding over 8 NeuronCores: core c owns (batch c//2, head-half c%2).
Each core computes attention for 8 heads of one batch and a partial
output projection; the host sums the two half-head partials per batch
and adds the bias.

Per-core layouts (partition dim first):
    xT/ctxT  [1024, 2048]  feature-major (host pre-transposes)
    Q^T/K^T  [512, 2048]   head-feature-major
    V_aug    [2048, 8*65]  row-major V with a ones column per head
    S^T      [m, n] tiles  -> exp -> P^T (float32r)
    O^T_aug = V_aug^T @ P^T  (row 64 of each head block = softmax sums)
    out     = O^T_all^T @ W_o  in natural [n, c] layout

All matmuls run in float32r (reduced-precision fp32, ~1.5e-4 rel err).
"""
import numpy as np

B, N_CTX, DIM = 4, 2048, 1024
HEADS, DHEAD = 16, 64
HHALF = HEADS // 2          # 8 heads per core
FDIM = HHALF * DHEAD        # 512 per-core qkv feature dim
N_CORES = 8
NCHUNK = 512                # free-dim chunk for matmuls / psum banks
KTILES = DIM // 128         # 8 contraction chunks for projections
FTILES = FDIM // 128        # 4 partition tiles of Q^T/K^T
MTILES = N_CTX // 128       # 16 row tiles
NCHUNKS = N_CTX // NCHUNK   # 4

ROW_PACK_B = False          # pack 2 heads into PE row bands for S^T matmuls

_CACHED = {}


def _build():
    import concourse.bass as bass
    import concourse.tile as tile
    from concourse import bacc, mybir

    f32 = mybir.dt.float32
    f32r = mybir.dt.float32r
    AF = mybir.ActivationFunctionType

    nc = bacc.Bacc("TRN2", target_bir_lowering=False, debug=False,
                   enable_asserts=True, num_devices=N_CORES)

    xT = nc.dram_tensor("xT", [DIM, N_CTX], f32, kind="ExternalInput").ap()
    ctxT = nc.dram_tensor("ctxT", [DIM, N_CTX], f32, kind="ExternalInput").ap()
    wq = nc.dram_tensor("wq", [DIM, FDIM], f32, kind="ExternalInput").ap()
    wk = nc.dram_tensor("wk", [DIM, FDIM], f32, kind="ExternalInput").ap()
    wv = nc.dram_tensor("wv", [DIM, FDIM], f32, kind="ExternalInput").ap()
    wo = nc.dram_tensor("wo", [FDIM, DIM], f32, kind="ExternalInput").ap()
    out = nc.dram_tensor("out", [N_CTX, DIM], f32, kind="ExternalOutput").ap()

    with tile.TileContext(nc) as tc:
        with (
            tc.tile_pool(name="qt", bufs=1) as qt_pool,
            tc.tile_pool(name="kt", bufs=1) as kt_pool,
            tc.tile_pool(name="vs", bufs=1) as vs_pool,
            tc.tile_pool(name="ot", bufs=1) as ot_pool,
        ):
            QT = [qt_pool.tile([128, N_CTX], f32r, name=f"QT{i}") for i in range(FTILES)]
            KT = [kt_pool.tile([128, N_CTX], f32r, name=f"KT{i}") for i in range(FTILES)]
            # V with a ones column appended per head: [128, 8*65]
            VA = [vs_pool.tile([128, HHALF * (DHEAD + 1)], f32r, name=f"VA{i}")
                  for i in range(MTILES)]
            OT = [ot_pool.tile([128, N_CTX], f32r, name=f"OT{i}") for i in range(FTILES)]

            # ---- stage A1: K^T and V from ctxT (streamed in 256-wide m chunks) ----
            MC = 256
            with (
                tc.tile_pool(name="ctxp", bufs=2) as ctxp,
                tc.tile_pool(name="wkp", bufs=1) as wkp,
                tc.tile_pool(name="wvp", bufs=1) as wvp,
                tc.tile_pool(name="psA1", bufs=4, space="PSUM") as psA1,
            ):
                WK = [wkp.tile([128, FDIM], f32r, name=f"WK{i}") for i in range(KTILES)]
                WV = [wvp.tile([128, FDIM], f32r, name=f"WV{i}") for i in range(KTILES)]
                for i in range(KTILES):
                    nc.gpsimd.dma_start(WK[i][:], wk[i * 128:(i + 1) * 128, :])
                    nc.gpsimd.dma_start(WV[i][:], wv[i * 128:(i + 1) * 128, :])

                for j in range(N_CTX // MC):
                    m0 = j * MC
                    CT = [ctxp.tile([128, MC], f32r, name=f"CT{k}_{j}", tag=f"ct{k}")
                          for k in range(KTILES)]
                    for k in range(KTILES):
                        nc.gpsimd.dma_start(CT[k][:], ctxT[k * 128:(k + 1) * 128, m0:m0 + MC])
                    # K^T[f, m0:m0+MC] = sum_k WK[k, f]^T ctxT[k, m]
                    for f in range(FTILES):
                        ps = psA1.tile([128, MC], f32, name=f"psK{f}_{j}", tag="psA1")
                        for k in range(KTILES):
                            nc.tensor.matmul(
                                ps[:], WK[k][:, f * 128:(f + 1) * 128], CT[k][:],
                                start=(k == 0), stop=(k == KTILES - 1))
                        nc.vector.tensor_copy(KT[f][:, m0:m0 + MC], ps[:])
                    # V rows m0..m0+MC: two 128-row tiles
                    for t in range(MC // 128):
                        m = j * (MC // 128) + t
                        ps = psA1.tile([128, FDIM], f32, name=f"psV{m}", tag="psA1v")
                        for k in range(KTILES):
                            nc.tensor.matmul(
                                ps[:], CT[k][:, t * 128:(t + 1) * 128], WV[k][:],
                                start=(k == 0), stop=(k == KTILES - 1))
                        # ones everywhere first, then V over the d-columns:
                        # head h cols [h*64, h*64+64) -> [h*65, h*65+64); col h*65+64 stays 1.0
                        nc.vector.memset(VA[m][:].bitcast(mybir.dt.uint32), 0x3F800000)
                        src = ps[:].rearrange("p (h d) -> p h d", h=HHALF)
                        dst = VA[m][:].rearrange("p (h e) -> p h e", h=HHALF)[:, :, 0:DHEAD]
                        nc.vector.tensor_copy(dst, src)

            # ---- stage A2: Q^T from xT (streamed in 512-wide n chunks) ----
            with (
                tc.tile_pool(name="xp", bufs=2) as xp,
                tc.tile_pool(name="wqp", bufs=1) as wqp,
                tc.tile_pool(name="psA2", bufs=4, space="PSUM") as psA2,
            ):
                WQ = [wqp.tile([128, FDIM], f32r, name=f"WQ{i}") for i in range(KTILES)]
                for i in range(KTILES):
                    nc.gpsimd.dma_start(WQ[i][:], wq[i * 128:(i + 1) * 128, :])
                for j in range(NCHUNKS):
                    n0 = j * NCHUNK
                    XT = [xp.tile([128, NCHUNK], f32r, name=f"XT{k}_{j}", tag=f"xt{k}")
                          for k in range(KTILES)]
                    for k in range(KTILES):
                        nc.gpsimd.dma_start(XT[k][:], xT[k * 128:(k + 1) * 128, n0:n0 + NCHUNK])
                    for f in range(FTILES):
                        ps = psA2.tile([128, NCHUNK], f32, name=f"psQ{f}_{j}", tag="psA2")
                        for k in range(KTILES):
                            nc.tensor.matmul(
                                ps[:], WQ[k][:, f * 128:(f + 1) * 128], XT[k][:],
                                start=(k == 0), stop=(k == KTILES - 1))
                        nc.vector.tensor_copy(QT[f][:, n0:n0 + NCHUNK], ps[:])

            # ---- stage B/C: per-head attention ----
            with (
                tc.tile_pool(name="pt", bufs=4) as pt_pool,
                tc.tile_pool(name="nrm", bufs=4) as nrm_pool,
                tc.tile_pool(name="psS", bufs=4, space="PSUM") as psS,
                tc.tile_pool(name="psO", bufs=2, space="PSUM") as psO,
            ):
                for h in range(HHALF):
                    ft, row = h // 2, (h % 2) * 64
                    kt_h = KT[ft][row:row + 64, :]
                    qt_h = QT[ft][row:row + 64, :]
                    for j in range(NCHUNKS):
                        po = psO.tile([DHEAD + 1, NCHUNK], f32, name=f"po{h}_{j}", tag="psO")
                        for m in range(MTILES):
                            ps = psS.tile([128, NCHUNK], f32,
                                          name=f"psS{h}_{j}_{m}", tag="psS")
                            nc.tensor.matmul(
                                ps[:], kt_h[:, m * 128:(m + 1) * 128],
                                qt_h[:, j * NCHUNK:(j + 1) * NCHUNK],
                                start=True, stop=True)
                            pt = pt_pool.tile([128, NCHUNK], f32r,
                                              name=f"pt{h}_{j}_{m}", tag="pt")
                            nc.scalar.activation(pt[:], ps[:], AF.Exp, scale=0.125)
                            nc.tensor.matmul(
                                po[:], VA[m][:, h * 65:h * 65 + 65], pt[:],
                                start=(m == 0), stop=(m == MTILES - 1))
                        # normalize: rows 0..63 are O^T, row 64 is sum
                        inv = nrm_pool.tile([1, NCHUNK], f32, name=f"inv{h}_{j}", tag="inv")
                        nc.vector.reciprocal(inv[:], po[DHEAD:DHEAD + 1, :])
                        bc = nrm_pool.tile([64, NCHUNK], f32, name=f"bc{h}_{j}", tag="bc")
                        nc.gpsimd.partition_broadcast(bc[:], inv[:])
                        nc.vector.tensor_tensor(
                            OT[ft][row:row + 64, j * NCHUNK:(j + 1) * NCHUNK],
                            po[0:DHEAD, :], bc[:], op=mybir.AluOpType.mult)

            # ---- stage D: out[n, c] = sum_f OT[f, n]^T W_o[f, c] ----
            with (
                tc.tile_pool(name="wop", bufs=1) as wop,
                tc.tile_pool(name="stg", bufs=4) as stg,
                tc.tile_pool(name="psD", bufs=4, space="PSUM") as psD,
            ):
                WO = [wop.tile([128, DIM], f32r, name=f"WO{i}") for i in range(FTILES)]
                for i in range(FTILES):
                    nc.gpsimd.dma_start(WO[i][:], wo[i * 128:(i + 1) * 128, :])
                for nt in range(MTILES):
                    for c in range(DIM // NCHUNK):
                        ps = psD.tile([128, NCHUNK], f32, name=f"psD{nt}_{c}", tag="psD")
                        for f in range(FTILES):
                            nc.tensor.matmul(
                                ps[:], OT[f][:, nt * 128:(nt + 1) * 128],
                                WO[f][:, c * NCHUNK:(c + 1) * NCHUNK],
                                start=(f == 0), stop=(f == FTILES - 1))
                        so = stg.tile([128, NCHUNK], f32, name=f"so{nt}_{c}", tag="so")
                        nc.vector.tensor_copy(so[:], ps[:])
                        nc.sync.dma_start(
                            out[nt * 128:(nt + 1) * 128, c * NCHUNK:(c + 1) * NCHUNK],
                            so[:])
    nc.compile()
    return nc


def _get_nc():
    if "nc" not in _CACHED:
        _CACHED["nc"] = _build()
    return _CACHED["nc"]


def _make_in_maps(inputs):
    x = np.asarray(inputs["x"], dtype=np.float32)
    context = np.asarray(inputs["context"], dtype=np.float32)
    W_q = np.asarray(inputs["W_q"], dtype=np.float32)
    W_k = np.asarray(inputs["W_k"], dtype=np.float32)
    W_v = np.asarray(inputs["W_v"], dtype=np.float32)
    W_o = np.asarray(inputs["W_o"], dtype=np.float32)

    xTs = [np.ascontiguousarray(x[b].T) for b in range(B)]
    cTs = [np.ascontiguousarray(context[b].T) for b in range(B)]
    in_maps = []
    for c in range(N_CORES):
        b, g = c // 2, c % 2
        cols = slice(g * FDIM, (g + 1) * FDIM)
        in_maps.append({
            "xT": xTs[b],
            "ctxT": cTs[b],
            "wq": np.ascontiguousarray(W_q[:, cols]),
            "wk": np.ascontiguousarray(W_k[:, cols]),
            "wv": np.ascontiguousarray(W_v[:, cols]),
            "wo": np.ascontiguousarray(W_o[g * FDIM:(g + 1) * FDIM, :]),
        })
    return in_maps


def kernel(x, context, W_q, W_k, W_v, W_o, b_o):
    from concourse.bass_utils import run_bass_kernel_spmd

    nc = _get_nc()
    b_o = np.asarray(b_o, dtype=np.float32)
    in_maps = _make_in_maps({
        "x": x, "context": context, "W_q": W_q, "W_k": W_k,
        "W_v": W_v, "W_o": W_o,
    })
    res = run_bass_kernel_spmd(nc, in_maps, list(range(N_CORES)))
    outp = np.empty((B, N_CTX, DIM), dtype=np.float32)
    for b in range(B):
        outp[b] = res.results[2 * b]["out"] + res.results[2 * b + 1]["out"] + b_o
    return outp


# revision 9
# speedup vs baseline: 1.4979x; 1.4979x over previous
"""Trainium2 Bass kernel for multi-head cross-attention.

Reference computation (fp32):
    q = (x @ W_q), k = (ctx @ W_k), v = (ctx @ W_v)   # per-head d=64, 16 heads
    out = softmax(q k^T / 8) v  concat-heads @ W_o + b_o

Sharding over 8 NeuronCores: core c owns (batch c//2, head-half c%2).
Each core computes attention for 8 heads of one batch and a partial
output projection; the host sums the two half-head partials per batch
and adds the bias.

Per-core layouts (partition dim first):
    xT/ctxT  [1024, 2048]  feature-major (host pre-transposes)
    Q^T/K^T  [512, 2048]   head-feature-major
    V_aug    [2048, 8*65]  row-major V with a ones column per head
    S^T      [m, n] tiles  -> exp -> P^T (float32r)
    O^T_aug = V_aug^T @ P^T  (row 64 of each head block = softmax sums)
    out     = O^T_all^T @ W_o  in natural [n, c] layout

All matmuls run in float32r (reduced-precision fp32, ~1.5e-4 rel err).
Head pairs share PE row bands (K=64 row packing) and a single
[128, 1024] exp activation per m-tile.
"""
import numpy as np

B, N_CTX, DIM = 4, 2048, 1024
HEADS, DHEAD = 16, 64
HHALF = HEADS // 2          # 8 heads per core
FDIM = HHALF * DHEAD        # 512 per-core qkv feature dim
N_CORES = 8
NCHUNK = 512                # free-dim chunk for matmuls / psum banks
KTILES = DIM // 128         # 8 contraction chunks for projections
FTILES = FDIM // 128        # 4 partition tiles of Q^T/K^T
MTILES = N_CTX // 128       # 16 row tiles
NCHUNKS = N_CTX // NCHUNK   # 4

_CACHED = {}


def _build():
    import concourse.bass as bass
    import concourse.tile as tile
    from concourse import bacc, mybir

    f32 = mybir.dt.float32
    f32r = mybir.dt.float32r
    AF = mybir.ActivationFunctionType

    nc = bacc.Bacc("TRN2", target_bir_lowering=False, debug=False,
                   enable_asserts=True, num_devices=N_CORES)

    xT = nc.dram_tensor("xT", [DIM, N_CTX], f32, kind="ExternalInput").ap()
    ctxT = nc.dram_tensor("ctxT", [DIM, N_CTX], f32, kind="ExternalInput").ap()
    wq = nc.dram_tensor("wq", [DIM, FDIM], f32, kind="ExternalInput").ap()
    wk = nc.dram_tensor("wk", [DIM, FDIM], f32, kind="ExternalInput").ap()
    wv = nc.dram_tensor("wv", [DIM, FDIM], f32, kind="ExternalInput").ap()
    wo = nc.dram_tensor("wo", [FDIM, DIM], f32, kind="ExternalInput").ap()
    out = nc.dram_tensor("out", [N_CTX, DIM], f32, kind="ExternalOutput").ap()

    def make_load_cast(stage_pool, stage_tag, stage_shape):
        def load_cast(pool, name, src, shape, tag=None):
            """HWDGE fp32 load into shared staging, DVE cast to float32r."""
            st = stage_pool.tile(stage_shape, f32, name=f"st_{name}", tag=stage_tag)
            nc.sync.dma_start(st[:shape[0], :shape[1]], src)
            t = pool.tile(shape, f32r, name=name, tag=tag)
            nc.vector.tensor_copy(t[:], st[:shape[0], :shape[1]])
            return t
        return load_cast

    with tile.TileContext(nc) as tc:
        with (
            tc.tile_pool(name="kt", bufs=1) as kt_pool,
            tc.tile_pool(name="vs", bufs=1) as vs_pool,
            tc.tile_pool(name="ot", bufs=1) as ot_pool,
        ):
            KT = [kt_pool.tile([128, N_CTX], f32r, name=f"KT{i}") for i in range(FTILES)]
            # V with a ones column appended per head: [128, 8*65]
            VA = [vs_pool.tile([128, HHALF * (DHEAD + 1)], f32r, name=f"VA{i}")
                  for i in range(MTILES)]
            OT = [ot_pool.tile([128, N_CTX], f32r, name=f"OT{i}") for i in range(FTILES)]

            # ---- stage A1: K^T and V from ctxT (streamed in 512-wide m chunks) ----
            MC = 512
            with (
                tc.tile_pool(name="ctxp", bufs=2) as ctxp,
                tc.tile_pool(name="stgA", bufs=4) as stgA,
                tc.tile_pool(name="wkp", bufs=1) as wkp,
                tc.tile_pool(name="wvp", bufs=1) as wvp,
                tc.tile_pool(name="psA1", bufs=4, space="PSUM") as psA1,
            ):
                load_cast = make_load_cast(stgA, "stg", [128, MC])
                WK = [load_cast(wkp, f"WK{i}", wk[i * 128:(i + 1) * 128, :], [128, FDIM])
                      for i in range(KTILES)]
                WV = [load_cast(wvp, f"WV{i}", wv[i * 128:(i + 1) * 128, :], [128, FDIM])
                      for i in range(KTILES)]

                for j in range(N_CTX // MC):
                    m0 = j * MC
                    CT = [load_cast(ctxp, f"CT{k}_{j}",
                                    ctxT[k * 128:(k + 1) * 128, m0:m0 + MC],
                                    [128, MC], tag=f"ct{k}") for k in range(KTILES)]
                    # K^T[f, m0:m0+MC] = sum_k WK[k, f]^T ctxT[k, m]
                    for f in range(FTILES):
                        ps = psA1.tile([128, MC], f32, name=f"psK{f}_{j}", tag="psA1")
                        for k in range(KTILES):
                            nc.tensor.matmul(
                                ps[:], WK[k][:, f * 128:(f + 1) * 128], CT[k][:],
                                start=(k == 0), stop=(k == KTILES - 1))
                        nc.vector.tensor_copy(KT[f][:, m0:m0 + MC], ps[:])
                    # V rows m0..m0+MC: four 128-row tiles
                    for t in range(MC // 128):
                        m = j * (MC // 128) + t
                        ps = psA1.tile([128, FDIM], f32, name=f"psV{m}", tag="psA1v")
                        for k in range(KTILES):
                            nc.tensor.matmul(
                                ps[:], CT[k][:, t * 128:(t + 1) * 128], WV[k][:],
                                start=(k == 0), stop=(k == KTILES - 1))
                        # ones everywhere first, then V over the d-columns:
                        # head h cols [h*64, h*64+64) -> [h*65, h*65+64)
                        nc.vector.memset(VA[m][:].bitcast(mybir.dt.uint32), 0x3F800000)
                        src = ps[:].rearrange("p (h d) -> p h d", h=HHALF)
                        dst = VA[m][:].rearrange("p (h e) -> p h e", h=HHALF)[:, :, 0:DHEAD]
                        nc.vector.tensor_copy(dst, src)

            with tc.tile_pool(name="qt", bufs=1) as qt_pool:
                QT = [qt_pool.tile([128, N_CTX], f32r, name=f"QT{i}")
                      for i in range(FTILES)]

                # ---- stage A2: Q^T from xT (streamed in 512-wide n chunks) ----
                with (
                    tc.tile_pool(name="xp", bufs=2) as xp,
                    tc.tile_pool(name="stgB", bufs=4) as stgB,
                    tc.tile_pool(name="wqp", bufs=1) as wqp,
                    tc.tile_pool(name="psA2", bufs=4, space="PSUM") as psA2,
                ):
                    load_cast = make_load_cast(stgB, "stg", [128, NCHUNK])
                    WQ = [load_cast(wqp, f"WQ{i}", wq[i * 128:(i + 1) * 128, :],
                                    [128, FDIM]) for i in range(KTILES)]
                    for j in range(NCHUNKS):
                        n0 = j * NCHUNK
                        XTt = [load_cast(xp, f"XT{k}_{j}",
                                         xT[k * 128:(k + 1) * 128, n0:n0 + NCHUNK],
                                         [128, NCHUNK], tag=f"xt{k}")
                               for k in range(KTILES)]
                        for f in range(FTILES):
                            ps = psA2.tile([128, NCHUNK], f32, name=f"psQ{f}_{j}", tag="psA2")
                            for k in range(KTILES):
                                nc.tensor.matmul(
                                    ps[:], WQ[k][:, f * 128:(f + 1) * 128], XTt[k][:],
                                    start=(k == 0), stop=(k == KTILES - 1))
                            nc.vector.tensor_copy(QT[f][:, n0:n0 + NCHUNK], ps[:])

                # ---- stage B/C: attention, one head-pair (= one QT/KT tile) at a time ----
                with (
                    tc.tile_pool(name="pt", bufs=3) as pt_pool,
                    tc.tile_pool(name="nrm", bufs=4) as nrm_pool,
                    tc.tile_pool(name="psS", bufs=2, space="PSUM") as psS,
                    tc.tile_pool(name="psO", bufs=4, space="PSUM") as psO,
                ):
                    for hp in range(FTILES):        # head pair (2hp, 2hp+1)
                        for jp in range(NCHUNKS // 2):
                            # two n-chunks per pass; 2 heads x 2 chunks of O^T psum
                            po = {}
                            for jj in range(2):
                                for hh in range(2):
                                    po[hh, jj] = psO.tile(
                                        [DHEAD + 1, NCHUNK], f32,
                                        name=f"po{hp}_{jp}_{hh}_{jj}", tag="psO")
                            for m in range(MTILES):
                                pts = []
                                for jj in range(2):
                                    j = 2 * jp + jj
                                    ps = psS.tile([128, 2 * NCHUNK], f32,
                                                  name=f"psS{hp}_{jp}_{m}_{jj}", tag="psS")
                                    # row-packed pair: head A in array rows 0-63,
                                    # head B in rows 64-127, separate psum banks
                                    for hh in range(2):
                                        r = hh * 64
                                        nc.tensor.matmul(
                                            ps[:, hh * NCHUNK:(hh + 1) * NCHUNK],
                                            KT[hp][r:r + 64, m * 128:(m + 1) * 128],
                                            QT[hp][r:r + 64, j * NCHUNK:(j + 1) * NCHUNK],
                                            start=True, stop=True)
                                    pt = pt_pool.tile([128, 2 * NCHUNK], f32r,
                                                      name=f"pt{hp}_{jp}_{m}_{jj}", tag="pt")
                                    nc.scalar.activation(pt[:], ps[:], AF.Exp, scale=0.125)
                                    pts.append(pt)
                                for jj in range(2):
                                    for hh in range(2):
                                        h = 2 * hp + hh
                                        nc.tensor.matmul(
                                            po[hh, jj][:],
                                            VA[m][:, h * 65:h * 65 + 65],
                                            pts[jj][:, hh * NCHUNK:(hh + 1) * NCHUNK],
                                            start=(m == 0), stop=(m == MTILES - 1))
                            # normalize: rows 0..63 are O^T, row 64 is the softmax sum
                            for jj in range(2):
                                j = 2 * jp + jj
                                for hh in range(2):
                                    p = po[hh, jj]
                                    srow = nrm_pool.tile(
                                        [1, NCHUNK], f32,
                                        name=f"srow{hp}_{jp}_{hh}_{jj}", tag="srow")
                                    nc.vector.tensor_copy(srow[:], p[DHEAD:DHEAD + 1, :])
                                    scr = nrm_pool.tile(
                                        [1, NCHUNK], f32,
                                        name=f"scr{hp}_{jp}_{hh}_{jj}", tag="scr")
                                    inv = nrm_pool.tile(
                                        [1, NCHUNK], f32,
                                        name=f"inv{hp}_{jp}_{hh}_{jj}", tag="inv")
                                    nc.vector.reciprocal_approx_accurate(
                                        inv[:], srow[:], scr[:])
                                    bc = nrm_pool.tile(
                                        [64, NCHUNK], f32,
                                        name=f"bc{hp}_{jp}_{hh}_{jj}", tag="bc")
                                    nc.gpsimd.partition_broadcast(bc[:], inv[:])
                                    nc.vector.tensor_tensor(
                                        OT[hp][hh * 64:hh * 64 + 64,
                                               j * NCHUNK:(j + 1) * NCHUNK],
                                        p[0:DHEAD, :], bc[:], op=mybir.AluOpType.mult)

                # ---- stage D: out[n, c] = sum_f OT[f, n]^T W_o[f, c] ----
                with (
                    tc.tile_pool(name="wop", bufs=1) as wop,
                    tc.tile_pool(name="stgC", bufs=2) as stgC,
                    tc.tile_pool(name="stg", bufs=4) as stg,
                    tc.tile_pool(name="psD", bufs=4, space="PSUM") as psD,
                ):
                    load_cast = make_load_cast(stgC, "stgc", [128, DIM])
                    WO = [load_cast(wop, f"WO{i}", wo[i * 128:(i + 1) * 128, :],
                                    [128, DIM]) for i in range(FTILES)]
                    for nt in range(MTILES):
                        for c in range(DIM // NCHUNK):
                            ps = psD.tile([128, NCHUNK], f32, name=f"psD{nt}_{c}", tag="psD")
                            for f in range(FTILES):
                                nc.tensor.matmul(
                                    ps[:], OT[f][:, nt * 128:(nt + 1) * 128],
                                    WO[f][:, c * NCHUNK:(c + 1) * NCHUNK],
                                    start=(f == 0), stop=(f == FTILES - 1))
                            so = stg.tile([128, NCHUNK], f32, name=f"so{nt}_{c}", tag="so")
                            nc.vector.tensor_copy(so[:], ps[:])
                            nc.sync.dma_start(
                                out[nt * 128:(nt + 1) * 128, c * NCHUNK:(c + 1) * NCHUNK],
                                so[:])
    nc.compile()
    return nc


def _get_nc():
    if "nc" not in _CACHED:
        _CACHED["nc"] = _build()
    return _CACHED["nc"]


def _make_in_maps(inputs):
    x = np.asarray(inputs["x"], dtype=np.float32)
    context = np.asarray(inputs["context"], dtype=np.float32)
    W_q = np.asarray(inputs["W_q"], dtype=np.float32)
    W_k = np.asarray(inputs["W_k"], dtype=np.float32)
    W_v = np.asarray(inputs["W_v"], dtype=np.float32)
    W_o = np.asarray(inputs["W_o"], dtype=np.float32)

    xTs = [np.ascontiguousarray(x[b].T) for b in range(B)]
    cTs = [np.ascontiguousarray(context[b].T) for b in range(B)]
    in_maps = []
    for c in range(N_CORES):
        b, g = c // 2, c % 2
        cols = slice(g * FDIM, (g + 1) * FDIM)
        in_maps.append({
            "xT": xTs[b],
            "ctxT": cTs[b],
            "wq": np.ascontiguousarray(W_q[:, cols]),
            "wk": np.ascontiguousarray(W_k[:, cols]),
            "wv": np.ascontiguousarray(W_v[:, cols]),
            "wo": np.ascontiguousarray(W_o[g * FDIM:(g + 1) * FDIM, :]),
        })
    return in_maps


def kernel(x, context, W_q, W_k, W_v, W_o, b_o):
    from concourse.bass_utils import run_bass_kernel_spmd

    nc = _get_nc()
    b_o = np.asarray(b_o, dtype=np.float32)
    in_maps = _make_in_maps({
        "x": x, "context": context, "W_q": W_q, "W_k": W_k,
        "W_v": W_v, "W_o": W_o,
    })
    res = run_bass_kernel_spmd(nc, in_maps, list(range(N_CORES)))
    outp = np.empty((B, N_CTX, DIM), dtype=np.float32)
    for b in range(B):
        outp[b] = res.results[2 * b]["out"] + res.results[2 * b + 1]["out"] + b_o
    return outp


# revision 11
# speedup vs baseline: 1.6754x; 1.1185x over previous
"""Trainium2 Bass kernel for multi-head cross-attention.

Reference computation (fp32):
    q = (x @ W_q), k = (ctx @ W_k), v = (ctx @ W_v)   # per-head d=64, 16 heads
    out = softmax(q k^T / 8) v  concat-heads @ W_o + b_o

Sharding over 8 NeuronCores: core c owns (batch c//2, head-half c%2).
Each core computes attention for 8 heads of one batch and a partial
output projection; the host sums the two half-head partials per batch
and adds the bias.

Per-core layouts (partition dim first):
    xT/ctxT  [1024, 2048]  feature-major (host pre-transposes)
    Q^T/K^T  [512, 2048]   head-feature-major
    V_aug    [2048, 8*65]  row-major V with a ones column per head
    S^T      [m, n] tiles  -> exp -> P^T (float32r)
    O^T_aug = V_aug^T @ P^T  (row 64 of each head block = softmax sums)
    out     = O^T_all^T @ W_o  in natural [n, c] layout

All matmuls run in float32r (reduced-precision fp32, ~1.5e-4 rel err).
Head pairs share PE row bands (explicit tile_position row packing) and a
single [128, 1024] exp activation per m-tile.
"""
import numpy as np

B, N_CTX, DIM = 4, 2048, 1024
HEADS, DHEAD = 16, 64
HHALF = HEADS // 2          # 8 heads per core
FDIM = HHALF * DHEAD        # 512 per-core qkv feature dim
N_CORES = 8
NCHUNK = 512                # free-dim chunk for matmuls / psum banks
KTILES = DIM // 128         # 8 contraction chunks for projections
FTILES = FDIM // 128        # 4 partition tiles of Q^T/K^T
MTILES = N_CTX // 128       # 16 row tiles
NCHUNKS = N_CTX // NCHUNK   # 4

_CACHED = {}


def _build():
    import concourse.bass as bass
    import concourse.tile as tile
    from concourse import bacc, mybir

    f32 = mybir.dt.float32
    f32r = mybir.dt.float32r
    AF = mybir.ActivationFunctionType

    nc = bacc.Bacc("TRN2", target_bir_lowering=False, debug=False,
                   enable_asserts=True, num_devices=N_CORES)

    xT = nc.dram_tensor("xT", [DIM, N_CTX], f32, kind="ExternalInput").ap()
    ctxT = nc.dram_tensor("ctxT", [DIM, N_CTX], f32, kind="ExternalInput").ap()
    wq = nc.dram_tensor("wq", [DIM, FDIM], f32, kind="ExternalInput").ap()
    wk = nc.dram_tensor("wk", [DIM, FDIM], f32, kind="ExternalInput").ap()
    wv = nc.dram_tensor("wv", [DIM, FDIM], f32, kind="ExternalInput").ap()
    wo = nc.dram_tensor("wo", [FDIM, DIM], f32, kind="ExternalInput").ap()
    out = nc.dram_tensor("out", [N_CTX, DIM], f32, kind="ExternalOutput").ap()

    with tile.TileContext(nc) as tc:
        with (
            tc.tile_pool(name="kt", bufs=1) as kt_pool,
            tc.tile_pool(name="vs", bufs=1) as vs_pool,
            tc.tile_pool(name="qt", bufs=1) as qt_pool,
        ):
            KT = [kt_pool.tile([128, N_CTX], f32r, name=f"KT{i}") for i in range(FTILES)]
            # V with a ones column appended per head: [128, 8*65]
            VA = [vs_pool.tile([128, HHALF * (DHEAD + 1)], f32r, name=f"VA{i}")
                  for i in range(MTILES)]
            QT = [qt_pool.tile([128, N_CTX], f32r, name=f"QT{i}") for i in range(FTILES)]

            # ---- stage A: projections, one shared streaming scope ----
            with (
                tc.tile_pool(name="chp", bufs=2) as chp,     # 512-wide input chunks
                tc.tile_pool(name="stgA", bufs=4) as stgA,   # fp32 DMA staging
                tc.tile_pool(name="psA", bufs=4, space="PSUM") as psA,
            ):
                def load_cast(pool, name, src, shape, tag=None):
                    st = stgA.tile([128, NCHUNK], f32, name=f"st_{name}", tag="stg")
                    nc.sync.dma_start(st[:shape[0], :shape[1]], src)
                    t = pool.tile(shape, f32r, name=name, tag=tag)
                    nc.vector.tensor_copy(t[:], st[:shape[0], :shape[1]])
                    return t

                # -- A1: K^T and V from ctxT --
                with (
                    tc.tile_pool(name="wkp", bufs=1) as wkp,
                    tc.tile_pool(name="wvp", bufs=1) as wvp,
                ):
                    WK = [load_cast(wkp, f"WK{i}", wk[i * 128:(i + 1) * 128, :],
                                    [128, FDIM]) for i in range(KTILES)]
                    WV = [load_cast(wvp, f"WV{i}", wv[i * 128:(i + 1) * 128, :],
                                    [128, FDIM]) for i in range(KTILES)]
                    for j in range(NCHUNKS):
                        m0 = j * NCHUNK
                        CT = [load_cast(chp, f"CT{k}_{j}",
                                        ctxT[k * 128:(k + 1) * 128, m0:m0 + NCHUNK],
                                        [128, NCHUNK], tag=f"ch{k}")
                              for k in range(KTILES)]
                        # K^T[f, m-chunk] = sum_k WK[k, f]^T ctxT[k, m]
                        for f in range(FTILES):
                            ps = psA.tile([128, NCHUNK], f32, name=f"psK{f}_{j}", tag="psA")
                            for k in range(KTILES):
                                nc.tensor.matmul(
                                    ps[:], WK[k][:, f * 128:(f + 1) * 128], CT[k][:],
                                    start=(k == 0), stop=(k == KTILES - 1))
                            nc.vector.tensor_copy(KT[f][:, m0:m0 + NCHUNK], ps[:])
                        # V rows m-chunk: four 128-row tiles
                        for t in range(NCHUNK // 128):
                            m = j * (NCHUNK // 128) + t
                            ps = psA.tile([128, FDIM], f32, name=f"psV{m}", tag="psA")
                            for k in range(KTILES):
                                nc.tensor.matmul(
                                    ps[:], CT[k][:, t * 128:(t + 1) * 128], WV[k][:],
                                    start=(k == 0), stop=(k == KTILES - 1))
                            # ones everywhere first, then V over the d-columns:
                            # head h cols [h*64, h*64+64) -> [h*65, h*65+64)
                            nc.vector.memset(VA[m][:].bitcast(mybir.dt.uint32), 0x3F800000)
                            src = ps[:].rearrange("p (h d) -> p h d", h=HHALF)
                            dst = VA[m][:].rearrange("p (h e) -> p h e", h=HHALF)[:, :, 0:DHEAD]
                            nc.vector.tensor_copy(dst, src)

                # -- A2: Q^T from xT (WQ reuses WK/WV space, chunks reuse chp) --
                with tc.tile_pool(name="wqp", bufs=1) as wqp:
                    WQ = [load_cast(wqp, f"WQ{i}", wq[i * 128:(i + 1) * 128, :],
                                    [128, FDIM]) for i in range(KTILES)]
                    for j in range(NCHUNKS):
                        n0 = j * NCHUNK
                        XTt = [load_cast(chp, f"XT{k}_{j}",
                                         xT[k * 128:(k + 1) * 128, n0:n0 + NCHUNK],
                                         [128, NCHUNK], tag=f"ch{k}")
                               for k in range(KTILES)]
                        for f in range(FTILES):
                            ps = psA.tile([128, NCHUNK], f32, name=f"psQ{f}_{j}", tag="psA")
                            for k in range(KTILES):
                                nc.tensor.matmul(
                                    ps[:], WQ[k][:, f * 128:(f + 1) * 128], XTt[k][:],
                                    start=(k == 0), stop=(k == KTILES - 1))
                            nc.vector.tensor_copy(QT[f][:, n0:n0 + NCHUNK], ps[:])

            # ---- stage B/C + D: OT reuses the stage-A streaming space ----
            with tc.tile_pool(name="ot", bufs=1) as ot_pool:
                OT = [ot_pool.tile([128, N_CTX], f32r, name=f"OT{i}")
                      for i in range(FTILES)]

                with (
                    tc.tile_pool(name="pt", bufs=3) as pt_pool,
                    tc.tile_pool(name="nrm", bufs=2) as nrm_pool,
                    tc.tile_pool(name="psS", bufs=2, space="PSUM") as psS,
                    tc.tile_pool(name="psO", bufs=4, space="PSUM") as psO,
                ):
                    for hp in range(FTILES):        # head pair (2hp, 2hp+1)
                        for j in range(NCHUNKS):
                            po = {}
                            for hh in range(2):
                                po[hh] = psO.tile(
                                    [DHEAD + 1, NCHUNK], f32,
                                    name=f"po{hp}_{j}_{hh}", tag="psO")
                            for m in range(MTILES):
                                ps = psS.tile([128, 2 * NCHUNK], f32,
                                              name=f"psS{hp}_{j}_{m}", tag="psS")
                                # row-packed pair: head A in array rows 0-63,
                                # head B in rows 64-127, separate psum banks
                                for hh in range(2):
                                    r = hh * 64
                                    nc.tensor.matmul(
                                        ps[:, hh * NCHUNK:(hh + 1) * NCHUNK],
                                        KT[hp][r:r + 64, m * 128:(m + 1) * 128],
                                        QT[hp][r:r + 64, j * NCHUNK:(j + 1) * NCHUNK],
                                        start=True, stop=True,
                                        tile_position=(r, 0))
                                pt = pt_pool.tile([128, 2 * NCHUNK], f32r,
                                                  name=f"pt{hp}_{j}_{m}", tag="pt")
                                nc.scalar.activation(pt[:], ps[:], AF.Exp, scale=0.125)
                                for hh in range(2):
                                    h = 2 * hp + hh
                                    nc.tensor.matmul(
                                        po[hh][:],
                                        VA[m][:, h * 65:h * 65 + 65],
                                        pt[:, hh * NCHUNK:(hh + 1) * NCHUNK],
                                        start=(m == 0), stop=(m == MTILES - 1))
                            # normalize: rows 0..63 are O^T, row 64 = softmax sum
                            for hh in range(2):
                                p = po[hh]
                                srow = nrm_pool.tile(
                                    [1, NCHUNK], f32,
                                    name=f"srow{hp}_{j}_{hh}", tag="srow")
                                nc.vector.tensor_copy(srow[:], p[DHEAD:DHEAD + 1, :])
                                scr = nrm_pool.tile(
                                    [1, NCHUNK], f32,
                                    name=f"scr{hp}_{j}_{hh}", tag="scr")
                                inv = nrm_pool.tile(
                                    [1, NCHUNK], f32,
                                    name=f"inv{hp}_{j}_{hh}", tag="inv")
                                nc.vector.reciprocal_approx_accurate(
                                    inv[:], srow[:], scr[:])
                                bc = nrm_pool.tile(
                                    [64, NCHUNK], f32,
                                    name=f"bc{hp}_{j}_{hh}", tag="bc")
                                nc.gpsimd.partition_broadcast(bc[:], inv[:])
                                nc.vector.tensor_tensor(
                                    OT[hp][hh * 64:hh * 64 + 64,
                                           j * NCHUNK:(j + 1) * NCHUNK],
                                    p[0:DHEAD, :], bc[:], op=mybir.AluOpType.mult)

                # ---- stage D: out[n, c] = sum_f OT[f, n]^T W_o[f, c] ----
                with (
                    tc.tile_pool(name="wop", bufs=1) as wop,
                    tc.tile_pool(name="stgD", bufs=2) as stgD,
                    tc.tile_pool(name="stg", bufs=4) as stg,
                    tc.tile_pool(name="psD", bufs=4, space="PSUM") as psD,
                ):
                    WO = []
                    for i in range(FTILES):
                        t = wop.tile([128, DIM], f32r, name=f"WO{i}")
                        for half in range(2):
                            st = stgD.tile([128, NCHUNK], f32,
                                           name=f"stwo{i}_{half}", tag="stgd")
                            nc.sync.dma_start(
                                st[:], wo[i * 128:(i + 1) * 128,
                                          half * NCHUNK:(half + 1) * NCHUNK])
                            nc.vector.tensor_copy(
                                t[:, half * NCHUNK:(half + 1) * NCHUNK], st[:])
                        WO.append(t)
                    for nt in range(MTILES):
                        for c in range(DIM // NCHUNK):
                            ps = psD.tile([128, NCHUNK], f32,
                                          name=f"psD{nt}_{c}", tag="psD")
                            for f in range(FTILES):
                                nc.tensor.matmul(
                                    ps[:], OT[f][:, nt * 128:(nt + 1) * 128],
                                    WO[f][:, c * NCHUNK:(c + 1) * NCHUNK],
                                    start=(f == 0), stop=(f == FTILES - 1))
                            so = stg.tile([128, NCHUNK], f32,
                                          name=f"so{nt}_{c}", tag="so")
                            nc.vector.tensor_copy(so[:], ps[:])
                            nc.sync.dma_start(
                                out[nt * 128:(nt + 1) * 128,
                                    c * NCHUNK:(c + 1) * NCHUNK],
                                so[:])
    nc.compile()
    return nc


def _get_nc():
    if "nc" not in _CACHED:
        _CACHED["nc"] = _build()
    return _CACHED["nc"]


def _make_in_maps(inputs):
    x = np.asarray(inputs["x"], dtype=np.float32)
    context = np.asarray(inputs["context"], dtype=np.float32)
    W_q = np.asarray(inputs["W_q"], dtype=np.float32)
    W_k = np.asarray(inputs["W_k"], dtype=np.float32)
    W_v = np.asarray(inputs["W_v"], dtype=np.float32)
    W_o = np.asarray(inputs["W_o"], dtype=np.float32)

    xTs = [np.ascontiguousarray(x[b].T) for b in range(B)]
    cTs = [np.ascontiguousarray(context[b].T) for b in range(B)]
    in_maps = []
    for c in range(N_CORES):
        b, g = c // 2, c % 2
        cols = slice(g * FDIM, (g + 1) * FDIM)
        in_maps.append({
            "xT": xTs[b],
            "ctxT": cTs[b],
            "wq": np.ascontiguousarray(W_q[:, cols]),
            "wk": np.ascontiguousarray(W_k[:, cols]),
            "wv": np.ascontiguousarray(W_v[:, cols]),
            "wo": np.ascontiguousarray(W_o[g * FDIM:(g + 1) * FDIM, :]),
        })
    return in_maps


def kernel(x, context, W_q, W_k, W_v, W_o, b_o):
    from concourse.bass_utils import run_bass_kernel_spmd

    nc = _get_nc()
    b_o = np.asarray(b_o, dtype=np.float32)
    in_maps = _make_in_maps({
        "x": x, "context": context, "W_q": W_q, "W_k": W_k,
        "W_v": W_v, "W_o": W_o,
    })
    res = run_bass_kernel_spmd(nc, in_maps, list(range(N_CORES)))
    outp = np.empty((B, N_CTX, DIM), dtype=np.float32)
    for b in range(B):
        outp[b] = res.results[2 * b]["out"] + res.results[2 * b + 1]["out"] + b_o
    return outp
